# revision 22
# baseline (speedup 1.0000x reference)
"""AxialAttention kernel for 8 Trainium2 NeuronCores.

Math (B=2, S=2048, E=1024, H=16, D=64):
  Qf = query @ Wq.T + bq ; Kf = query @ Wk.T (+bk dropped: softmax row-shift
  invariant) ; Vf = query @ Wv.T (+bv folded in post-attn: softmax rows sum
  to 1).
  mean-over-heads of per-head scores == (Qf @ Kf.T) / (H*sqrt(D))  [full-E
  contraction].
  P = softmax(scores) ; A = P @ Vf + bv
  Reference reshape quirk: Y[s', e'] = A[q, e] with s' = h*128 + q//16,
  e' = (q%16)*64 + d  (e = h*64+d).  out = Y @ Wo.T + bo.

Sharding: 8 cores = 2 batches x 4 query-row quarters (512 rows each).
Each core computes full Kf/Vf for its batch (duplicated x4, no
collectives), its 512-row slice of everything else, and 512 permuted
output rows which the host scatters. bo added on host.
"""

import numpy as np

B, S, E, H, D = 2, 2048, 1024, 16, 64
SQ = 512          # query rows per core
NE = E // 128     # 8 e-tiles
NK = S // 128     # 16 k-tiles
NCORES = 8

_CACHE = {}


def _build(mm_f32r=True):
    from contextlib import ExitStack

    import concourse.bacc as bacc
    import concourse.tile as tile
    from concourse import mybir
    from concourse.masks import make_identity

    f32 = mybir.dt.float32
    f32r = mybir.dt.float32r

    mdt = f32r if mm_f32r else f32

    def mm(ap):
        return ap

    nc = bacc.Bacc(
        "TRN2", target_bir_lowering=False, debug=False, enable_asserts=True,
        num_devices=NCORES,
    )
    qf_d = nc.dram_tensor("qf", [S, E], f32r, kind="ExternalInput").ap()
    qft_d = nc.dram_tensor("qft", [E, S], f32r, kind="ExternalInput").ap()
    qpt_d = nc.dram_tensor("qpt", [E, SQ], f32r, kind="ExternalInput").ap()
    wq_d = nc.dram_tensor("wq", [E, E], f32r, kind="ExternalInput").ap()
    wk_d = nc.dram_tensor("wk", [E, E], f32r, kind="ExternalInput").ap()
    wvt_d = nc.dram_tensor("wvt", [E, E], f32r, kind="ExternalInput").ap()
    wot_d = nc.dram_tensor("wot", [E, E], f32r, kind="ExternalInput").ap()
    vt_d = nc.dram_tensor("vt", [E], f32, kind="ExternalInput").ap()
    bv_d = nc.dram_tensor("bv", [E], f32, kind="ExternalInput").ap()
    out_d = nc.dram_tensor("out", [SQ, E], f32, kind="ExternalOutput").ap()

    with tile.TileContext(nc) as tc:
        stack = ExitStack()
        with stack:
            ps = stack.enter_context(tc.tile_pool(name="ps", bufs=8, space="PSUM"))
            small = stack.enter_context(tc.tile_pool(name="small", bufs=1))
            stats = stack.enter_context(tc.tile_pool(name="stats", bufs=2))
            wn_pool = stack.enter_context(tc.tile_pool(name="wn", bufs=4))
            pt_pool = stack.enter_context(tc.tile_pool(name="pt", bufs=16))

            ident = small.tile([128, 128], f32)
            make_identity(nc, ident)
            vT = small.tile([128, NE], f32)
            nc.sync.dma_start(out=vT, in_=vt_d.rearrange("(t p) -> p t", p=128))
            bvT = small.tile([128, NE], f32)
            nc.sync.dma_start(out=bvT, in_=bv_d.rearrange("(t p) -> p t", p=128))

            # ---- Phase A: resident query (f32r) + q_partT --------------------
            qfn_cm = tc.tile_pool(name="qfn", bufs=16)
            qfn_pool = qfn_cm.__enter__()
            t2t_cm = tc.tile_pool(name="t2t", bufs=8)
            t2t_pool = t2t_cm.__enter__()
            t2t = [t2t_pool.tile([128, SQ], mdt, tag="t2t", name=f"t2t{i}")
                   for i in range(NE)]
            qpt_cm = tc.tile_pool(name="qpt", bufs=8)
            qpt_pool = qpt_cm.__enter__()
            qpt = [qpt_pool.tile([128, SQ], mdt, tag="qpt", name=f"qpt{i}")
                   for i in range(NE)]
            for i in range(NE):
                nc.sync.dma_start(out=qpt[i],
                                  in_=qpt_d[i * 128:(i + 1) * 128, :])

            # ---- Phase B': G = (Wq.T @ Wk)/128 from natural layouts;
            #      v = (bq @ Wk)/128 ; t2T = (qp @ G).T -------------------------
            g_cm = tc.tile_pool(name="g", bufs=8)
            g_pool = g_cm.__enter__()
            g = [g_pool.tile([128, E], mdt, tag="g", name=f"g{i}")
                 for i in range(NE)]
            for eh in range(2):  # e'' halves; 8 psum banks per pass
                gps = [ps.tile([128, SQ], f32, tag="ps", name=f"gps{i}")
                       for i in range(NE)]
                for f in range(NE):
                    wqn = wn_pool.tile([128, E], f32r, tag="wn",
                                       name=f"wqn{eh}_{f}")
                    nc.sync.dma_start(out=wqn,
                                      in_=wq_d[f * 128:(f + 1) * 128, :])
                    wkn = wn_pool.tile([128, SQ], f32r, tag="wn",
                                       name=f"wkn{eh}_{f}")
                    nc.sync.dma_start(
                        out=wkn, in_=wk_d[f * 128:(f + 1) * 128,
                                          eh * 512:(eh + 1) * 512])
                    for eo in range(NE):
                        nc.tensor.matmul(
                            gps[eo], wqn[:, eo * 128:(eo + 1) * 128], wkn,
                            start=(f == 0), stop=(f == NE - 1),
                            skip_group_check=True)
                for eo in range(NE):
                    nc.scalar.activation(
                        out=g[eo][:, eh * 512:(eh + 1) * 512], in_=gps[eo],
                        func=mybir.ActivationFunctionType.Copy, scale=1.0 / 128.0)

            for es in range(NE):
                mps = ps.tile([128, SQ], f32, tag="ps", name="t2ps")
                for eo in range(NE):
                    nc.tensor.matmul(
                        mps, g[eo][:, es * 128:(es + 1) * 128], qpt[eo],
                        start=(eo == 0), stop=(eo == NE - 1))
                nc.vector.tensor_scalar_add(out=t2t[es], in0=mps,
                                            scalar1=vT[:, es:es + 1])
            g_cm.__exit__(None, None, None)
            qpt_cm.__exit__(None, None, None)

            # ---- Phase C': scores = t2 @ qf.T  [SQ, S], q4-outer so softmax
            #      and P-transposes pipeline into the scores phase.
            #      queryT lives in the qfn pool slots, later reused for the
            #      natural query tiles (WAR handled by Tile).
            qtb2 = [qfn_pool.tile([128, E], mdt, tag="qfn", name=f"qtb2_{i}")
                    for i in range(NK)]
            for et in range(NE):
                for half in range(2):
                    nc.sync.dma_start(
                        out=qtb2[et * 2 + half],
                        in_=qft_d[et * 128:(et + 1) * 128,
                                  half * 1024:(half + 1) * 1024])
            sc_cm = tc.tile_pool(name="sc", bufs=4)
            sc_pool = sc_cm.__enter__()
            sc = [sc_pool.tile([128, S], f32, tag="sc", name=f"sc{i}")
                  for i in range(4)]
            for q4 in range(4):
                for kb in range(4):
                    sps = ps.tile([128, SQ], f32, tag="ps", name="sps")
                    for es in range(NE):
                        nc.tensor.matmul(
                            sps, t2t[es][:, q4 * 128:(q4 + 1) * 128],
                            qtb2[es * 2 + kb // 2][:, (kb % 2) * 512:
                                                   (kb % 2 + 1) * 512],
                            start=(es == 0), stop=(es == NE - 1))
                    nc.scalar.activation(
                        out=sc[q4][:, kb * 512:(kb + 1) * 512], in_=sps,
                        func=mybir.ActivationFunctionType.Copy, scale=1.0)

            # ---- Phase D: softmax rows, in place -----------------------------
            for q4 in range(4):
                rm = stats.tile([128, 1], f32, tag="rm", name="rm")
                nc.vector.reduce_max(out=rm, in_=sc[q4], axis=mybir.AxisListType.X)
                nm = stats.tile([128, 1], f32, tag="nm", name="nm")
                nc.vector.tensor_scalar_mul(out=nm, in0=rm, scalar1=-1.0)
                rs = stats.tile([128, 1], f32, tag="rs", name="rs")
                nc.scalar.activation(
                    out=sc[q4], in_=sc[q4], func=mybir.ActivationFunctionType.Exp,
                    bias=nm[:, 0:1], scale=1.0, accum_out=rs[:, 0:1])
                ri = stats.tile([128, 1], f32, tag="ri", name="ri")
                nc.vector.reciprocal(out=ri, in_=rs)
                nc.vector.tensor_scalar_mul(out=sc[q4], in0=sc[q4],
                                            scalar1=ri[:, 0:1])

            # ---- Phase E: PT = P.T  [S, SQ] ----------------------------------
            pt = [pt_pool.tile([128, SQ], mdt, tag="pt", name=f"pt{i}")
                  for i in range(NK)]
            for q4 in range(4):
                for kt in range(NK):
                    tps = ps.tile([128, 128], f32, tag="ps", name="tps2")
                    nc.tensor.transpose(tps, sc[q4][:, kt * 128:(kt + 1) * 128],
                                        ident)
                    nc.any.tensor_copy(out=pt[kt][:, q4 * 128:(q4 + 1) * 128],
                                       in_=tps)
            sc_cm.__exit__(None, None, None)
            t2t_cm.__exit__(None, None, None)

            # ---- Phase F': R0 = P @ qf ; AT = (R0 @ Wv.T).T + bv -----------
            wt_cm = tc.tile_pool(name="wt", bufs=8)
            wt_pool = wt_cm.__enter__()
            at_cm = tc.tile_pool(name="at", bufs=8)
            at_pool = at_cm.__enter__()
            at = [at_pool.tile([128, SQ], f32, tag="at", name=f"at{i}")
                  for i in range(NE)]
            r0t_cm = tc.tile_pool(name="r0t", bufs=8)
            r0t_pool = r0t_cm.__enter__()
            r0t = [r0t_pool.tile([128, SQ], mdt, tag="r0t", name=f"r0t{i}")
                   for i in range(NE)]
            # R0T computed directly: lhsT=qfn slice, rhs=PT
            qfn = [qfn_pool.tile([128, E], f32r, tag="qfn", name=f"qfn{i}")
                   for i in range(NK)]
            for kt in range(NK):
                nc.sync.dma_start(out=qfn[kt],
                                  in_=qf_d[kt * 128:(kt + 1) * 128, :])
            rps = [ps.tile([128, SQ], f32, tag="ps", name=f"rps{i}")
                   for i in range(NE)]
            for kt in range(NK):
                for es in range(NE):
                    nc.tensor.matmul(
                        rps[es], qfn[kt][:, es * 128:(es + 1) * 128], pt[kt],
                        start=(kt == 0), stop=(kt == NK - 1),
                        skip_group_check=True)
            for es in range(NE):
                nc.any.tensor_copy(out=r0t[es], in_=rps[es])
            # WvT
            wt3 = [wt_pool.tile([128, E], mdt, tag="wt", name=f"wtv{i}")
                   for i in range(NE)]
            for i in range(NE):
                nc.sync.dma_start(out=wt3[i],
                                  in_=wvt_d[i * 128:(i + 1) * 128, :])
            # AT = WvT.T-slices @ R0T  (= (R0 @ Wv.T).T), + bv
            for eo in range(NE):
                mps = ps.tile([128, SQ], f32, tag="ps", name="aps")
                for et in range(NE):
                    nc.tensor.matmul(
                        mps, wt3[et][:, eo * 128:(eo + 1) * 128], r0t[et],
                        start=(et == 0), stop=(et == NE - 1))
                nc.vector.tensor_scalar_add(out=at[eo], in0=mps,
                                            scalar1=bvT[:, eo:eo + 1])
            r0t_cm.__exit__(None, None, None)

            # ---- Phase G: YT[e', c] = AT permuted ----------------------------
            # YT[(q2,d), h*32+q1] = AT[(h%2)*64+d of tile h//2, q1*16+q2]
            yt_cm = tc.tile_pool(name="yt", bufs=8)
            yt_pool = yt_cm.__enter__()
            yt = [yt_pool.tile([128, SQ], mdt, tag="yt", name=f"yt{i}")
                  for i in range(NE)]
            for t in range(NE):
                for h in range(H):
                    src = at[h // 2].rearrange("p (q1 q2) -> p q2 q1", q2=16)
                    for a2 in range(2):
                        nc.any.tensor_copy(
                            out=yt[t][a2 * 64:(a2 + 1) * 64, h * 32:(h + 1) * 32],
                            in_=src[(h % 2) * 64:(h % 2) * 64 + 64, 2 * t + a2, :])

            # ---- Phase H: out = YT.T @ WoT  [SQ, E]  (bo added on host) ------
            wt4 = [wt_pool.tile([128, E], mdt, tag="wt", name=f"wto{i}")
                   for i in range(NE)]
            for i in range(NE):
                nc.sync.dma_start(out=wt4[i],
                                  in_=wot_d[i * 128:(i + 1) * 128, :])
            on_cm = tc.tile_pool(name="on", bufs=2)
            on_pool = on_cm.__enter__()
            for st4 in range(4):
                on = on_pool.tile([128, E], f32, tag="on", name="on")
                for nb in range(2):
                    mps = ps.tile([128, SQ], f32, tag="ps", name="ops")
                    for et in range(NE):
                        nc.tensor.matmul(
                            mps, mm(yt[et][:, st4 * 128:(st4 + 1) * 128]),
                            mm(wt4[et][:, nb * 512:(nb + 1) * 512]),
                            start=(et == 0), stop=(et == NE - 1))
                    nc.any.tensor_copy(out=on[:, nb * 512:(nb + 1) * 512], in_=mps)
                nc.sync.dma_start(out=out_d[st4 * 128:(st4 + 1) * 128, :], in_=on)
            on_cm.__exit__(None, None, None)
            yt_cm.__exit__(None, None, None)
            at_cm.__exit__(None, None, None)
            wt_cm.__exit__(None, None, None)
            qfn_cm.__exit__(None, None, None)

    nc.compile()
    return nc


def _get_nc():
    if "nc" not in _CACHE:
        _CACHE["nc"] = _build()
    return _CACHE["nc"]


def _in_maps(query, Wq, bq, Wk, bk, Wv, bv, Wo, bo):
    f = np.float32
    qft_by_batch = [np.ascontiguousarray(query[b].T, dtype=f)
                    for b in range(B)]
    wvt = np.ascontiguousarray(np.asarray(Wv, f).T)
    wot = np.ascontiguousarray(np.asarray(Wo, f).T)
    vt = np.ascontiguousarray(
        (np.asarray(bq, f) @ np.asarray(Wk, f)) / 128.0, dtype=f)
    maps = []
    for c in range(NCORES):
        b, r = divmod(c, 4)
        qftb = qft_by_batch[b]
        maps.append({
            "qf": np.ascontiguousarray(query[b], dtype=f),
            "qft": qftb,
            "qpt": np.ascontiguousarray(qftb[:, r * SQ:(r + 1) * SQ]),
            "wq": np.ascontiguousarray(Wq, dtype=f),
            "wk": np.ascontiguousarray(Wk, dtype=f),
            "wvt": wvt,
            "wot": wot,
            "vt": vt,
            "bv": np.ascontiguousarray(bv, dtype=f),
        })
    return maps


def _assemble(results, bo):
    out = np.empty((B, S, E), np.float32)
    for c in range(NCORES):
        b, r = divmod(c, 4)
        out[b].reshape(H, 128, E)[:, r * 32:(r + 1) * 32, :] = \
            results[c]["out"].reshape(H, 32, E)
    out += np.asarray(bo, np.float32)
    return out


def kernel(query, Wq, bq, Wk, bk, Wv, bv, Wo, bo):
    from concourse.bass_utils import run_bass_kernel_spmd
    nc = _get_nc()
    query = np.asarray(query, np.float32)
    maps = _in_maps(query, Wq, bq, Wk, bk, Wv, bv, Wo, bo)
    res = run_bass_kernel_spmd(nc, maps, core_ids=list(range(NCORES))).results
    return _assemble(res, bo)


def kernel_profiled(query, Wq, bq, Wk, bk, Wv, bv, Wo, bo, tmpdir=None):
    """Like kernel() but requests an NTFF trace; returns (out, BassKernelResults)."""
    from concourse.bass_utils import run_bass_kernel_spmd
    nc = _get_nc()
    query = np.asarray(query, np.float32)
    maps = _in_maps(query, Wq, bq, Wk, bk, Wv, bv, Wo, bo)
    r = run_bass_kernel_spmd(nc, maps, core_ids=list(range(NCORES)), trace=True,
                             tmpdir=tmpdir)
    return _assemble(r.results, bo), r


# revision 23
# speedup vs baseline: 1.0587x; 1.0587x over previous
"""AxialAttention kernel for 8 Trainium2 NeuronCores.

Math (B=2, S=2048, E=1024, H=16, D=64):
  Qf = query @ Wq.T + bq ; Kf = query @ Wk.T (+bk dropped: softmax row-shift
  invariant) ; Vf = query @ Wv.T (+bv folded in post-attn: softmax rows sum
  to 1).
  mean-over-heads of per-head scores == (Qf @ Kf.T) / (H*sqrt(D))  [full-E
  contraction].
  P = softmax(scores) ; A = P @ Vf + bv
  Reference reshape quirk: Y[s', e'] = A[q, e] with s' = h*128 + q//16,
  e' = (q%16)*64 + d  (e = h*64+d).  out = Y @ Wo.T + bo.

Sharding: 8 cores = 2 batches x 4 query-row quarters (512 rows each).
Each core computes full Kf/Vf for its batch (duplicated x4, no
collectives), its 512-row slice of everything else, and 512 permuted
output rows which the host scatters. bo added on host.
"""

import numpy as np

B, S, E, H, D = 2, 2048, 1024, 16, 64
SQ = 512          # query rows per core
NE = E // 128     # 8 e-tiles
NK = S // 128     # 16 k-tiles
NCORES = 8

_CACHE = {}


def _build(mm_f32r=True):
    from contextlib import ExitStack

    import concourse.bacc as bacc
    import concourse.tile as tile
    from concourse import mybir
    from concourse.masks import make_identity

    f32 = mybir.dt.float32
    f32r = mybir.dt.float32r

    mdt = f32r if mm_f32r else f32

    def mm(ap):
        return ap

    nc = bacc.Bacc(
        "TRN2", target_bir_lowering=False, debug=False, enable_asserts=True,
        num_devices=NCORES,
    )
    qf_d = nc.dram_tensor("qf", [S, E], f32r, kind="ExternalInput").ap()
    qft_d = nc.dram_tensor("qft", [E, S], f32r, kind="ExternalInput").ap()
    qpt_d = nc.dram_tensor("qpt", [E, SQ], f32r, kind="ExternalInput").ap()
    wq_d = nc.dram_tensor("wq", [E, E], f32r, kind="ExternalInput").ap()
    wk_d = nc.dram_tensor("wk", [E, E], f32r, kind="ExternalInput").ap()
    wvt_d = nc.dram_tensor("wvt", [E, E], f32r, kind="ExternalInput").ap()
    wot_d = nc.dram_tensor("wot", [E, E], f32r, kind="ExternalInput").ap()
    vt_d = nc.dram_tensor("vt", [E], f32, kind="ExternalInput").ap()
    bv_d = nc.dram_tensor("bv", [E], f32, kind="ExternalInput").ap()
    out_d = nc.dram_tensor("out", [SQ, E], f32, kind="ExternalOutput").ap()

    with tile.TileContext(nc) as tc:
        stack = ExitStack()
        with stack:
            ps = stack.enter_context(tc.tile_pool(name="ps", bufs=8, space="PSUM"))
            small = stack.enter_context(tc.tile_pool(name="small", bufs=1))
            stats = stack.enter_context(tc.tile_pool(name="stats", bufs=2))
            wn_pool = stack.enter_context(tc.tile_pool(name="wn", bufs=4))
            pt_pool = stack.enter_context(tc.tile_pool(name="pt", bufs=16))

            ident = small.tile([128, 128], f32)
            make_identity(nc, ident)
            vT = small.tile([128, NE], f32)
            nc.sync.dma_start(out=vT, in_=vt_d.rearrange("(t p) -> p t", p=128))
            bvT = small.tile([128, NE], f32)
            nc.sync.dma_start(out=bvT, in_=bv_d.rearrange("(t p) -> p t", p=128))

            # ---- Phase A: resident query (f32r) + q_partT --------------------
            qfn_cm = tc.tile_pool(name="qfn", bufs=16)
            qfn_pool = qfn_cm.__enter__()
            t2t_cm = tc.tile_pool(name="t2t", bufs=8)
            t2t_pool = t2t_cm.__enter__()
            t2t = [t2t_pool.tile([128, SQ], mdt, tag="t2t", name=f"t2t{i}")
                   for i in range(NE)]
            qpt_cm = tc.tile_pool(name="qpt", bufs=8)
            qpt_pool = qpt_cm.__enter__()
            qpt = [qpt_pool.tile([128, SQ], mdt, tag="qpt", name=f"qpt{i}")
                   for i in range(NE)]
            for i in range(NE):
                nc.sync.dma_start(out=qpt[i],
                                  in_=qpt_d[i * 128:(i + 1) * 128, :])

            # ---- Phase B': G = (Wq.T @ Wk)/128 from natural layouts;
            #      v = (bq @ Wk)/128 ; t2T = (qp @ G).T -------------------------
            g_cm = tc.tile_pool(name="g", bufs=8)
            g_pool = g_cm.__enter__()
            g = [g_pool.tile([128, E], mdt, tag="g", name=f"g{i}")
                 for i in range(NE)]
            wqr = [qfn_pool.tile([128, E], f32r, tag="qfn", name=f"wqr{i}")
                   for i in range(NE)]
            for f in range(NE):
                nc.sync.dma_start(out=wqr[f],
                                  in_=wq_d[f * 128:(f + 1) * 128, :])
            for eh in range(2):  # e'' halves; 8 psum banks per pass
                gps = [ps.tile([128, SQ], f32, tag="ps", name=f"gps{i}")
                       for i in range(NE)]
                for f in range(NE):
                    wkn = wn_pool.tile([128, SQ], f32r, tag="wn",
                                       name=f"wkn{eh}_{f}")
                    nc.sync.dma_start(
                        out=wkn, in_=wk_d[f * 128:(f + 1) * 128,
                                          eh * 512:(eh + 1) * 512])
                    for eo in range(NE):
                        nc.tensor.matmul(
                            gps[eo], wqr[f][:, eo * 128:(eo + 1) * 128], wkn,
                            start=(f == 0), stop=(f == NE - 1),
                            skip_group_check=True)
                for eo in range(NE):
                    nc.scalar.activation(
                        out=g[eo][:, eh * 512:(eh + 1) * 512], in_=gps[eo],
                        func=mybir.ActivationFunctionType.Copy, scale=1.0 / 128.0)

            for es in range(NE):
                mps = ps.tile([128, SQ], f32, tag="ps", name="t2ps")
                for eo in range(NE):
                    nc.tensor.matmul(
                        mps, g[eo][:, es * 128:(es + 1) * 128], qpt[eo],
                        start=(eo == 0), stop=(eo == NE - 1))
                nc.vector.tensor_scalar_add(out=t2t[es], in0=mps,
                                            scalar1=vT[:, es:es + 1])
            g_cm.__exit__(None, None, None)
            qpt_cm.__exit__(None, None, None)

            # ---- Phase C': scores = t2 @ qf.T  [SQ, S], q4-outer so softmax
            #      and P-transposes pipeline into the scores phase.
            #      queryT lives in the qfn pool slots, later reused for the
            #      natural query tiles (WAR handled by Tile).
            qtb2 = [qfn_pool.tile([128, E], mdt, tag="qfn", name=f"qtb2_{i}")
                    for i in range(NK)]
            for et in range(NE):
                for half in range(2):
                    nc.sync.dma_start(
                        out=qtb2[et * 2 + half],
                        in_=qft_d[et * 128:(et + 1) * 128,
                                  half * 1024:(half + 1) * 1024])
            sc_cm = tc.tile_pool(name="sc", bufs=4)
            sc_pool = sc_cm.__enter__()
            sc = [sc_pool.tile([128, S], f32, tag="sc", name=f"sc{i}")
                  for i in range(4)]
            for q4 in range(4):
                for kb in range(4):
                    sps = ps.tile([128, SQ], f32, tag="ps", name="sps")
                    for es in range(NE):
                        nc.tensor.matmul(
                            sps, t2t[es][:, q4 * 128:(q4 + 1) * 128],
                            qtb2[es * 2 + kb // 2][:, (kb % 2) * 512:
                                                   (kb % 2 + 1) * 512],
                            start=(es == 0), stop=(es == NE - 1))
                    nc.scalar.activation(
                        out=sc[q4][:, kb * 512:(kb + 1) * 512], in_=sps,
                        func=mybir.ActivationFunctionType.Copy, scale=1.0)

            # ---- Phase D: softmax rows, in place -----------------------------
            for q4 in range(4):
                rm = stats.tile([128, 1], f32, tag="rm", name="rm")
                nc.vector.reduce_max(out=rm, in_=sc[q4], axis=mybir.AxisListType.X)
                nm = stats.tile([128, 1], f32, tag="nm", name="nm")
                nc.vector.tensor_scalar_mul(out=nm, in0=rm, scalar1=-1.0)
                rs = stats.tile([128, 1], f32, tag="rs", name="rs")
                nc.scalar.activation(
                    out=sc[q4], in_=sc[q4], func=mybir.ActivationFunctionType.Exp,
                    bias=nm[:, 0:1], scale=1.0, accum_out=rs[:, 0:1])
                ri = stats.tile([128, 1], f32, tag="ri", name="ri")
                nc.vector.reciprocal(out=ri, in_=rs)
                nc.vector.tensor_scalar_mul(out=sc[q4], in0=sc[q4],
                                            scalar1=ri[:, 0:1])

            # ---- Phase E: PT = P.T  [S, SQ] ----------------------------------
            pt = [pt_pool.tile([128, SQ], mdt, tag="pt", name=f"pt{i}")
                  for i in range(NK)]
            for q4 in range(4):
                for kt in range(NK):
                    tps = ps.tile([128, 128], f32, tag="ps", name="tps2")
                    nc.tensor.transpose(tps, sc[q4][:, kt * 128:(kt + 1) * 128],
                                        ident)
                    nc.any.tensor_copy(out=pt[kt][:, q4 * 128:(q4 + 1) * 128],
                                       in_=tps)
            sc_cm.__exit__(None, None, None)
            t2t_cm.__exit__(None, None, None)

            # ---- Phase F': R0 = P @ qf ; AT = (R0 @ Wv.T).T + bv -----------
            wt_cm = tc.tile_pool(name="wt", bufs=8)
            wt_pool = wt_cm.__enter__()
            at_cm = tc.tile_pool(name="at", bufs=8)
            at_pool = at_cm.__enter__()
            at = [at_pool.tile([128, SQ], f32, tag="at", name=f"at{i}")
                  for i in range(NE)]
            r0t_cm = tc.tile_pool(name="r0t", bufs=8)
            r0t_pool = r0t_cm.__enter__()
            r0t = [r0t_pool.tile([128, SQ], mdt, tag="r0t", name=f"r0t{i}")
                   for i in range(NE)]
            # R0T computed directly: lhsT=qfn slice, rhs=PT
            qfn = [qfn_pool.tile([128, E], f32r, tag="qfn", name=f"qfn{i}")
                   for i in range(NK)]
            for kt in range(NK):
                nc.sync.dma_start(out=qfn[kt],
                                  in_=qf_d[kt * 128:(kt + 1) * 128, :])
            rps = [ps.tile([128, SQ], f32, tag="ps", name=f"rps{i}")
                   for i in range(NE)]
            for kt in range(NK):
                for es in range(NE):
                    nc.tensor.matmul(
                        rps[es], qfn[kt][:, es * 128:(es + 1) * 128], pt[kt],
                        start=(kt == 0), stop=(kt == NK - 1),
                        skip_group_check=True)
            for es in range(NE):
                nc.any.tensor_copy(out=r0t[es], in_=rps[es])
            # WvT
            wt3 = [wt_pool.tile([128, E], mdt, tag="wt", name=f"wtv{i}")
                   for i in range(NE)]
            for i in range(NE):
                nc.sync.dma_start(out=wt3[i],
                                  in_=wvt_d[i * 128:(i + 1) * 128, :])
            # AT = WvT.T-slices @ R0T  (= (R0 @ Wv.T).T), + bv
            for eo in range(NE):
                mps = ps.tile([128, SQ], f32, tag="ps", name="aps")
                for et in range(NE):
                    nc.tensor.matmul(
                        mps, wt3[et][:, eo * 128:(eo + 1) * 128], r0t[et],
                        start=(et == 0), stop=(et == NE - 1))
                nc.vector.tensor_scalar_add(out=at[eo], in0=mps,
                                            scalar1=bvT[:, eo:eo + 1])
            r0t_cm.__exit__(None, None, None)

            # ---- Phase G: YT[e', c] = AT permuted ----------------------------
            # YT[(q2,d), h*32+q1] = AT[(h%2)*64+d of tile h//2, q1*16+q2]
            yt_cm = tc.tile_pool(name="yt", bufs=8)
            yt_pool = yt_cm.__enter__()
            yt = [yt_pool.tile([128, SQ], mdt, tag="yt", name=f"yt{i}")
                  for i in range(NE)]
            for t in range(NE):
                for h in range(H):
                    src = at[h // 2].rearrange("p (q1 q2) -> p q2 q1", q2=16)
                    for a2 in range(2):
                        nc.any.tensor_copy(
                            out=yt[t][a2 * 64:(a2 + 1) * 64, h * 32:(h + 1) * 32],
                            in_=src[(h % 2) * 64:(h % 2) * 64 + 64, 2 * t + a2, :])

            # ---- Phase H: out = YT.T @ WoT  [SQ, E]  (bo added on host) ------
            wt4 = [wt_pool.tile([128, E], mdt, tag="wt", name=f"wto{i}")
                   for i in range(NE)]
            for i in range(NE):
                nc.sync.dma_start(out=wt4[i],
                                  in_=wot_d[i * 128:(i + 1) * 128, :])
            on_cm = tc.tile_pool(name="on", bufs=2)
            on_pool = on_cm.__enter__()
            for st4 in range(4):
                on = on_pool.tile([128, E], f32, tag="on", name="on")
                for nb in range(2):
                    mps = ps.tile([128, SQ], f32, tag="ps", name="ops")
                    for et in range(NE):
                        nc.tensor.matmul(
                            mps, mm(yt[et][:, st4 * 128:(st4 + 1) * 128]),
                            mm(wt4[et][:, nb * 512:(nb + 1) * 512]),
                            start=(et == 0), stop=(et == NE - 1))
                    nc.any.tensor_copy(out=on[:, nb * 512:(nb + 1) * 512], in_=mps)
                nc.sync.dma_start(out=out_d[st4 * 128:(st4 + 1) * 128, :], in_=on)
            on_cm.__exit__(None, None, None)
            yt_cm.__exit__(None, None, None)
            at_cm.__exit__(None, None, None)
            wt_cm.__exit__(None, None, None)
            qfn_cm.__exit__(None, None, None)

    nc.compile()
    return nc


def _get_nc():
    if "nc" not in _CACHE:
        _CACHE["nc"] = _build()
    return _CACHE["nc"]


def _in_maps(query, Wq, bq, Wk, bk, Wv, bv, Wo, bo):
    f = np.float32
    qft_by_batch = [np.ascontiguousarray(query[b].T, dtype=f)
                    for b in range(B)]
    wvt = np.ascontiguousarray(np.asarray(Wv, f).T)
    wot = np.ascontiguousarray(np.asarray(Wo, f).T)
    vt = np.ascontiguousarray(
        (np.asarray(bq, f) @ np.asarray(Wk, f)) / 128.0, dtype=f)
    maps = []
    for c in range(NCORES):
        b, r = divmod(c, 4)
        qftb = qft_by_batch[b]
        maps.append({
            "qf": np.ascontiguousarray(query[b], dtype=f),
            "qft": qftb,
            "qpt": np.ascontiguousarray(qftb[:, r * SQ:(r + 1) * SQ]),
            "wq": np.ascontiguousarray(Wq, dtype=f),
            "wk": np.ascontiguousarray(Wk, dtype=f),
            "wvt": wvt,
            "wot": wot,
            "vt": vt,
            "bv": np.ascontiguousarray(bv, dtype=f),
        })
    return maps


def _assemble(results, bo):
    out = np.empty((B, S, E), np.float32)
    for c in range(NCORES):
        b, r = divmod(c, 4)
        out[b].reshape(H, 128, E)[:, r * 32:(r + 1) * 32, :] = \
            results[c]["out"].reshape(H, 32, E)
    out += np.asarray(bo, np.float32)
    return out


def kernel(query, Wq, bq, Wk, bk, Wv, bv, Wo, bo):
    from concourse.bass_utils import run_bass_kernel_spmd
    nc = _get_nc()
    query = np.asarray(query, np.float32)
    maps = _in_maps(query, Wq, bq, Wk, bk, Wv, bv, Wo, bo)
    res = run_bass_kernel_spmd(nc, maps, core_ids=list(range(NCORES))).results
    return _assemble(res, bo)


def kernel_profiled(query, Wq, bq, Wk, bk, Wv, bv, Wo, bo, tmpdir=None):
    """Like kernel() but requests an NTFF trace; returns (out, BassKernelResults)."""
    from concourse.bass_utils import run_bass_kernel_spmd
    nc = _get_nc()
    query = np.asarray(query, np.float32)
    maps = _in_maps(query, Wq, bq, Wk, bk, Wv, bv, Wo, bo)
    r = run_bass_kernel_spmd(nc, maps, core_ids=list(range(NCORES)), trace=True,
                             tmpdir=tmpdir)
    return _assemble(r.results, bo), r
